# revision 59
# baseline (speedup 1.0000x reference)
"""CAMIL self-attention kernel for 8 Trainium2 NeuronCores.

Reference computation (per bag b of B=4, N=4096 instances, D=512 features):
    qk = x @ W_qk.T ; q, k = split(qk)          (att dim E=64)
    v  = x @ W_v.T
    logits_n = (1/8) * sum_m adj[n,m] * (q_n . k_m)
             = (q_n/8) . (adj @ k)_n
    w = softmax(logits over N) ; out = w * v

Sharding: 2 cores per bag, each core owns NH=2048 rows (query dim). The
adjacency is pre-rotated on host so every core's rows are local m-tiles
0..15; the softmax normalization is completed with a tiny AllReduce of the
local sumexp between the two cores of each bag.

Numerics / dataflow (all tuned against the TimelineSim cost model, where
DMA payloads serialize on one exclusive device at ~360 B/ns and matmul cost
is output-free-size x cycles-per-row):
- The core loads only its own half of x in bf16 (Q, K-near, V); the other
  half is needed only for K, which tolerates fp8, so it ships as
  single-plane fp8 e4m3 (half the bytes). The far K projection runs
  entirely in fp8 (W_k^T staged as fp8 hi+lo planes, scaled x32 so the
  planes stay in e4m3 normal range; the 1/32 is folded into W_q together
  with the 1/sqrt(E) attention scale).
- QK uses x as the stationary operand so q/k come out of PSUM row-major -
  no PE transposes. Both S2 halves accumulate into one PSUM supertile
  consumed by a handful of big DVE/ACT ops: per-op fixed SBUF/PSUM access
  latency (~250-370ns) would otherwise pace the pipeline at ~2x the matmul
  rate.
- K is split on-device into fp8 hi+lo planes packed side-by-side in the
  DoubleRow stationary operand so a single pass over the adjacency computes
  both planes into PSUM partitions 0:64 (hi) and 64:128 (lo) at the fp8
  DoubleRow rate. The adjacency is exact in fp8 (binary) and streams
  n-block-major (4 column blocks of 512 rows, 512KB sub-DMAs) so each
  block's logits, exp and sum-partial complete while later blocks are still
  in flight. V-projection matmuls pace 1:1 with the stream subs (4 run
  pre-stream) so the PE never falls behind the adjacency DMA.
- Logit dot + softmax in fp32 on DVE/ACT; a fixed shift exp(l - 80) keeps
  exp in fp32 range (logits ~N(0,20), per-bag max ~75-100) and removes the
  serial max-reduction from the critical path. The last block's tail is the
  exposed critical path: its PSUM read is one full-width DVE copy (measured
  faster than DVE+ACT or DVE+DVE half-splits: ACT dispatches ~0.9us late
  after an idle period, and split copies pay the fixed PSUM access latency
  twice while the z-matmuls gate on the last half anyway).
- w-scales all run on DVE (194ns vs 612ns on ACT) and the output is stored
  bf16 (half the store bytes, upcast to fp32 on host) in packs of
  [2,2,4,4,4] row-tiles so the first store launches after two scales while
  later, larger packs keep HWDGE issue off the critical path.
"""

import sys

sys.path.insert(0, "/opt/trn_rl_repo")

import numpy as np

import concourse.bass as bass
import concourse.tile as tile
from concourse import bacc, bass_isa, mybir
from concourse.bass_utils import run_bass_kernel_spmd
from concourse.masks import make_identity

B, N, D, E = 4, 4096, 512, 64
P = 128
NCORES = 8
NH = N // 2        # rows per core
TH = NH // P       # 16 row-tiles per core (own half)
DT = D // P        # 4 d-tiles
KSCALE = 32.0      # K-path scale so fp8 W_k planes stay in e4m3 normal range
F32 = mybir.dt.float32
BF16 = mybir.dt.bfloat16
F8 = mybir.dt.float8e4
DR = mybir.MatmulPerfMode.DoubleRow
GROUPS = [[0, 1], [2, 3], [4, 5], [6, 7]]
I16 = mybir.dt.int16
# sumexp exchange via SWDGE prepare/trigger was attempted (descriptors
# prepared mid-stream, fired with cheap Pool-SEQ triggers to skip the two
# ~1.3us HWDGE+DGE issue latencies on the softmax tail) but the Tile
# framework's swdge semaphore management deadlocks the Pool queue; keep the
# plain-DMACopy exchange.
CC_SWDGE = False


def _build(single=False):
    # single=True: replace the cross-core AllReduce with a local DMA so the
    # module has no collectives (for TimelineSim cost modeling only).
    nc = bacc.Bacc(
        "TRN2", target_bir_lowering=False, num_devices=NCORES,
        num_swdge_queues=2 if CC_SWDGE else 1,
    )

    xt = nc.dram_tensor("xt", [D, NH], BF16, kind="ExternalInput")
    xtf = nc.dram_tensor("xtf", [D, NH], F8, kind="ExternalInput")
    at = nc.dram_tensor("at", [N, NH], F8, kind="ExternalInput")
    # weights staged partition-major on host so each partition's row is one
    # contiguous >=512B run (sub-512B runs pay a 2x DMA penalty)
    wqkt = nc.dram_tensor("wqkt", [P, DT * 2 * E], BF16, kind="ExternalInput")
    wktf = nc.dram_tensor("wktf", [P, DT * 2 * E], F8, kind="ExternalInput")
    wvt = nc.dram_tensor("wvt", [D, D], BF16, kind="ExternalInput")
    if CC_SWDGE:
        # swdge index patterns: col 0 = scatter ([0, -1 x15]), cols 1:9 =
        # gather (128 zeros, one per destination partition)
        idxs = nc.dram_tensor("idxs", [16, 9], I16, kind="ExternalInput")
    out = nc.dram_tensor("out", [NH, D], BF16, kind="ExternalOutput")

    xt_v = xt.ap().rearrange("(o p) n -> p o n", p=P)        # [128, 4, 2048]
    xtf_v = xtf.ap().rearrange("(o p) n -> p o n", p=P)      # [128, 4, 2048]
    at_v = at.ap().rearrange("(mo p) n -> p mo n", p=P)      # [128, 32, 2048]
    wqkt_v = wqkt.ap().rearrange("p (o e) -> p o e", o=DT)   # [128, 4, 128]
    wktf_v = wktf.ap().rearrange("p (o l e) -> p o l e", o=DT, l=2)
    wvt_v = wvt.ap().rearrange("(o p) e -> p o e", p=P)      # [128, 4, 512]
    out_v = out.ap().rearrange("(t p) e -> p t e", p=P)      # [128, 16, 512]

    with tile.TileContext(nc) as tc:
        with tc.tile_pool(name="big", bufs=1) as big, \
             tc.tile_pool(name="atp", bufs=16) as atp, \
             tc.tile_pool(name="ostream", bufs=5) as ostream, \
             tc.tile_pool(name="small", bufs=2) as small, \
             tc.tile_pool(name="dram", bufs=1, space="DRAM") as dram:

            # ---- constants ----
            # stacked double identity [I64; I64]: rt.T @ dident fuses the
            # z^T transpose with the hi+lo plane sum in a single PE op
            dident = big.tile([P, E], BF16)
            make_identity(nc, dident[0:E, 0:E])
            make_identity(nc, dident[E:2 * E, 0:E])
            # touch Exp once so the ACT table load is off the softmax path
            warm = small.tile([1, 1], F32, tag="warm")
            nc.gpsimd.memset(warm[:], 0.0)
            nc.scalar.activation(
                warm[:], warm[:], mybir.ActivationFunctionType.Exp
            )
            LSHIFT = 80.0
            nshift = small.tile([P, 1], F32, tag="nshift")
            nc.gpsimd.memset(nshift[:], -LSHIFT)

            # ---- input DMAs, all issued up front on the SP queue ----
            wqkt_sb = big.tile([P, DT, 2 * E], BF16)
            nc.sync.dma_start(out=wqkt_sb[:], in_=wqkt_v)
            wktf_sb = big.tile([P, DT, 2, E], F8, tag="wktf")
            nc.sync.dma_start(out=wktf_sb[:], in_=wktf_v)

            xt_q = []
            for j in range(4):
                xq = big.tile([P, DT, 512], BF16, tag=f"xt_q{j}")
                nc.sync.dma_start(
                    out=xq[:], in_=xt_v[:, :, j * 512:(j + 1) * 512]
                )
                xt_q.append(xq)

            xf_h = []
            for j in range(2):
                xf = big.tile([P, DT, 1024], F8, tag=f"xf_h{j}")
                nc.sync.dma_start(
                    out=xf[:], in_=xtf_v[:, :, j * 1024:(j + 1) * 1024]
                )
                xf_h.append(xf)

            wvt_sb = big.tile([P, DT, D], BF16, tag="wvt")
            nc.sync.dma_start(out=wvt_sb[:], in_=wvt_v)

            if CC_SWDGE:
                # swdge sumexp-exchange staging: index patterns + zeroed DRAM
                # landing pads (scatter-ADD needs a zero base). Tiny DMAs,
                # issued before the at stream so they are done by tail time.
                idx_sb = small.tile([16, 9], I16, tag="idx_sb")
                nc.sync.dma_start(out=idx_sb[:], in_=idxs.ap())
                s_pad = small.tile([P, 64], F32, tag="s_pad")
                nc.gpsimd.memset(s_pad[:], 0.0)
                gath_pad = small.tile([P, 1, 64], F32, tag="gath_pad")
                zq = small.tile([1, 64], F32, tag="zq")
                nc.gpsimd.memset(zq[:], 0.0)
                cc_pad = dram.tile([1, 64], F32)
                cc_out_pad = dram.tile([1, 64], F32)
                nc.sync.dma_start(out=cc_pad[:], in_=zq[:])
                nc.sync.dma_start(out=cc_out_pad[:], in_=zq[:])

            # n-block-major adjacency stream: block j = all 32 m-tiles x
            # cols j*512..(j+1)*512, in 4 sub-DMAs of 8 m-tiles each. Each
            # block's logits complete while later blocks still stream.
            at_tiles = {}
            for j in range(4):
                for s in range(4):
                    # (splitting the last sub-DMA in two was tried to start
                    # its DoubleRow pairs earlier; the extra HWDGE issue made
                    # it a net loss)
                    t = atp.tile([P, 8, 512], F8, tag="at_t",
                                 name=f"at{j}_{s}")
                    nc.sync.dma_start(
                        out=t[:],
                        in_=at_v[:, 8 * s:8 * s + 8, j * 512:(j + 1) * 512],
                    )
                    at_tiles[(j, s)] = t

            # prepare the sumexp-exchange descriptors now (desc-gen runs on
            # Pool while the stream loads); the tail just fires triggers.
            if CC_SWDGE:
                cc_dsem = nc.alloc_semaphore("cc_dsem")
                bc_dsem = nc.alloc_semaphore("bc_dsem")
                nc.gpsimd.dma_scatter_add(
                    out_ap=cc_pad[:],
                    in_ap=s_pad[:].unsqueeze(1),
                    idxs_ap=idx_sb[:, 0:1],
                    num_idxs=16,
                    num_idxs_reg=16,
                    elem_size=64,
                    prepare_only=True,
                    sem=cc_dsem,
                    queue_num=0,
                )
                gath_src = cc_pad if single else cc_out_pad
                nc.gpsimd.dma_gather(
                    out_ap=gath_pad[:],
                    in_ap=gath_src[:],
                    idxs_ap=idx_sb[:, 1:9],
                    num_idxs=128,
                    num_idxs_reg=128,
                    elem_size=64,
                    prepare_only=True,
                    sem=bc_dsem,
                    queue_num=1,
                )

            q_nat = big.tile([P, TH, E], BF16)
            # K for all 32 m-tiles in one tile, packed (hi 0:E | lo E:2E) per
            # m-tile; tiles 0..15 = own half (bf16 path), 16..31 = far (fp8)
            k_all = big.tile([P, 2 * TH, 2 * E], F8, tag="k_all")
            v_ev = big.tile([P, TH // 2, D], BF16, tag="v_ev")
            v_od = big.tile([P, TH // 2, D], BF16, tag="v_od")
            rt_c = [
                big.tile([P, 512], BF16, tag=f"rt{rc}", name=f"rt{rc}")
                for rc in range(4)
            ]
            l_sb = big.tile([P, TH], F32)

            # ---- S2: QK projections, x stationary so q/k land row-major.
            # All 16 tiles of each half accumulate into one PSUM supertile,
            # consumed by a handful of big DVE/ACT ops: the per-op fixed
            # SBUF/PSUM access latency (~250-370ns) would otherwise pace the
            # whole pipeline at ~2x the matmul rate. ----
            with tc.tile_pool(name="ps_a", bufs=1, space="PSUM") as ps_a, \
                 tc.tile_pool(name="ps_b", bufs=1, space="PSUM") as ps_b:
                # near (own) half: bf16, fused Q|K -> [n, 128] per tile
                pqk = ps_a.tile([P, TH, 2 * E], F32)
                for t in range(TH):
                    xo = (t % 4) * P
                    for di in range(DT):
                        nc.tensor.matmul(
                            pqk[:, t, :],
                            xt_q[t // 4][:, di, xo:xo + P],
                            wqkt_sb[:, di, :],
                            start=(di == 0),
                            stop=(di == DT - 1),
                        )
                nc.scalar.copy(out=q_nat[:], in_=pqk[:, :, 0:E])
                nc.vector.tensor_copy(
                    out=k_all[:, 0:TH, 0:E], in_=pqk[:, :, E:2 * E]
                )
                nc.vector.tensor_tensor(
                    out=k_all[:, 0:TH, E:2 * E],
                    in0=pqk[:, :, E:2 * E],
                    in1=k_all[:, 0:TH, 0:E],
                    op=mybir.AluOpType.subtract,
                )
                # far half: all-fp8 K-only projection, W_k hi+lo planes
                pkf = ps_b.tile([P, TH, E], F32)
                for t in range(TH):
                    xo = (t % 8) * P
                    for di in range(DT):
                        for pl in range(2):
                            nc.tensor.matmul(
                                pkf[:, t, :],
                                xf_h[t // 8][:, di, xo:xo + P],
                                wktf_sb[:, di, pl, :],
                                start=(di == 0 and pl == 0),
                                stop=(di == DT - 1 and pl == 1),
                            )
                nc.scalar.copy(out=k_all[:, TH:2 * TH, 0:E], in_=pkf[:])
                nc.vector.tensor_tensor(
                    out=k_all[:, TH:2 * TH, E:2 * E],
                    in0=pkf[:],
                    in1=k_all[:, TH:2 * TH, 0:E],
                    op=mybir.AluOpType.subtract,
                )

            # ---- V projection interleaved 1:1 with S4 DoubleRow pairs ----
            # S4: R^T = (adj @ [K_hi K_lo])^T. lhsT [128, 2, 128]: m-tile
            # pair, cols = (hi 64 | lo 64); rhs [128, 2, 512]: same pair of
            # adjacency m-tiles. One pass over adj fills PSUM rows 0:64 (hi)
            # and 64:128 (lo). V-tile t fills the PE while at-tile t+1 lands.
            with tc.tile_pool(name="ps_r", bufs=1, space="PSUM") as ps_r, \
                 tc.tile_pool(name="ps_s", bufs=2, space="PSUM") as ps_s:
                with tc.tile_pool(name="ps_v", bufs=2, space="PSUM") as ps_v:
                    # one psum tile per n-block; each block's tail (psum
                    # read, z-dot, exp partial) runs while later blocks
                    # still stream in.
                    psum_rj = [
                        ps_r.tile([P, 512], F32, tag=f"pr{j}", name=f"pr{j}")
                        for j in range(4)
                    ]
                    e_sb = small.tile([P, TH], F32, tag="e_sb")
                    s_p4 = small.tile([P, 4], F32, tag="s_p4")
                    def v_tile(t):
                        psum_v = ps_v.tile([P, 512], F32, tag="pv",
                                           name=f"psv{t}")
                        xr = xt_q[t // 4]
                        xo = (t % 4) * P
                        for di in range(DT):
                            nc.tensor.matmul(
                                psum_v[:],
                                xr[:, di, xo:xo + P],
                                wvt_sb[:, di, :],
                                start=(di == 0),
                                stop=(di == DT - 1),
                            )
                        if t % 2 == 0:
                            nc.vector.tensor_copy(
                                out=v_ev[:, t // 2, :], in_=psum_v[:]
                            )
                        else:
                            nc.scalar.copy(out=v_od[:, t // 2, :], in_=psum_v[:])

                    # 4 V tiles run pre-stream (right after wvt lands), the
                    # remaining 12 pace 1:1 with the first 12 adjacency subs
                    # so the PE never falls behind the at stream.
                    for t in range(4):
                        v_tile(t)

                    for j in range(4):
                        for s in range(4):
                            si = 4 * j + s
                            if si < 12:
                                v_tile(4 + si)

                            a_t = at_tiles[(j, s)]
                            for gi in range(4):
                                g = 4 * s + gi
                                kt = k_all[:, 2 * g:2 * g + 2, :]
                                if isinstance(a_t, tuple):
                                    src = a_t[gi // 2][
                                        :, 2 * (gi % 2):2 * (gi % 2) + 2, :
                                    ]
                                else:
                                    src = a_t[:, 2 * gi:2 * gi + 2, :]
                                nc.tensor.matmul(
                                    psum_rj[j][:],
                                    kt,
                                    src,
                                    start=(s == 0 and gi == 0),
                                    stop=(s == 3 and gi == 3),
                                    perf_mode=DR,
                                    skip_group_check=True,
                                )

                        # ---- block-j tail: psum -> SBUF, z = hi+lo via
                        # double-identity matmul, l = q.z, exp partial ----
                        # split the psum read across DVE and ACT so the
                        # serial tail of the last block is halved
                        nc.vector.tensor_copy(
                            out=rt_c[j][:], in_=psum_rj[j][:]
                        )
                        zp4 = ps_s.tile([P, 4, E], F32, tag="ps",
                                        name=f"z5_{j}")
                        z4 = small.tile([P, 4, E], BF16, tag="z4",
                                        name=f"z4_{j}")
                        for i in range(4):
                            nc.tensor.matmul(
                                zp4[:, i, :],
                                rt_c[j][:, i * P:(i + 1) * P],
                                dident[:],
                                start=True,
                                stop=True,
                            )
                        nc.vector.tensor_tensor(
                            out=z4[:], in0=zp4[:],
                            in1=q_nat[:, j * 4:(j + 1) * 4, :],
                            op=mybir.AluOpType.mult,
                        )
                        nc.vector.tensor_reduce(
                            out=l_sb[:, j * 4:(j + 1) * 4], in_=z4[:],
                            axis=mybir.AxisListType.X, op=mybir.AluOpType.add,
                        )
                        nc.scalar.activation(
                            e_sb[:, j * 4:(j + 1) * 4],
                            l_sb[:, j * 4:(j + 1) * 4],
                            mybir.ActivationFunctionType.Exp,
                            bias=nshift[:, 0:1], scale=1.0,
                            accum_out=s_p4[:, j:j + 1],
                        )
                s_loc = small.tile([P, 1], F32, tag="s_loc")
                nc.vector.tensor_reduce(
                    out=s_loc[:], in_=s_p4[:],
                    axis=mybir.AxisListType.X, op=mybir.AluOpType.add,
                )
                s_red = small.tile([P, 1], F32, tag="s_red")
                nc.gpsimd.partition_all_reduce(
                    s_red[:], s_loc[:], channels=P,
                    reduce_op=bass_isa.ReduceOp.add,
                )

                s_inv = small.tile([P, 1], F32, tag="s_inv")
                if CC_SWDGE:
                    # scatter s_red -> DRAM, AllReduce, gather-broadcast back.
                    # Tile transfers the preps' data deps onto the triggers
                    # and attaches the SWDGE DMA sems to consumers itself.
                    nc.vector.tensor_copy(out=s_pad[:, 0:1], in_=s_red[:])
                    nc.gpsimd.trigger_dma(count=None, queue_num=0)
                    if not single:
                        nc.gpsimd.collective_compute(
                            "AllReduce",
                            mybir.AluOpType.add,
                            replica_groups=GROUPS,
                            ins=[cc_pad[0:1, 0:1].opt()],
                            outs=[cc_out_pad[0:1, 0:1].opt()],
                        )
                    nc.gpsimd.trigger_dma(count=None, queue_num=1)
                    nc.vector.reciprocal(s_inv[:], gath_pad[:, 0, 0:1])
                else:
                    cc_in = dram.tile([1, 1], F32)
                    cc_out = dram.tile([1, 1], F32)
                    nc.sync.dma_start(out=cc_in[:], in_=s_red[0:1, :])
                    if single:
                        cc_res = cc_in
                    else:
                        nc.gpsimd.collective_compute(
                            "AllReduce",
                            mybir.AluOpType.add,
                            replica_groups=GROUPS,
                            ins=[cc_in[:].opt()],
                            outs=[cc_out[:].opt()],
                        )
                        cc_res = cc_out
                    # broadcast-load the pair total to every partition
                    gath_bc = small.tile([P, 1], F32, tag="gath_bc")
                    nc.sync.dma_start(
                        out=gath_bc[:],
                        in_=cc_res[:].rearrange("a b -> (a b)").unsqueeze(0)
                        .broadcast_to((P, 1)),
                    )
                    nc.vector.reciprocal(s_inv[:], gath_bc[:])

                # w = exp(l - LSHIFT) / S
                w_sb = small.tile([P, TH], F32, tag="w_sb")
                nc.vector.tensor_scalar_mul(w_sb[:], e_sb[:], s_inv[:, 0:1])

                # ---- S7: scale V by w and store (bf16) ----
                # all scales on DVE (194ns each vs 612 on ACT); first store
                # pack is small so the store pipeline starts early, DVE then
                # stays ahead of the 1456ns/pack DMA pace
                PACKS = [2, 2, 4, 4, 4]
                t = 0
                for pi, pk in enumerate(PACKS):
                    o_t = ostream.tile(
                        [P, pk, D], BF16, tag=f"o_t{pk}", name=f"ot{pi}"
                    )
                    for i in range(pk):
                        src = (v_ev[:, (t + i) // 2, :] if (t + i) % 2 == 0
                               else v_od[:, (t + i) // 2, :])
                        nc.vector.tensor_scalar_mul(
                            o_t[:, i, :], src, w_sb[:, t + i:t + i + 1],
                        )
                    nc.sync.dma_start(
                        out=out_v[:, t:t + pk, :], in_=o_t[:]
                    )
                    t += pk

    nc.compile()
    return nc


def prepare_in_maps(x, adj_matrix, W_qk, W_v):
    import ml_dtypes

    x = np.asarray(x, dtype=np.float32)
    adj = np.asarray(adj_matrix, dtype=np.float32)
    wqkt = np.ascontiguousarray(np.asarray(W_qk, dtype=np.float32).T)
    wqkt = wqkt.copy()
    # fold attention scale and the 1/KSCALE K-plane scale into W_q
    wqkt[:, :E] *= 1.0 / (np.sqrt(E) * KSCALE)
    wqkt[:, E:] *= KSCALE
    # partition-major staging: element (p, o, e) = wqkt[o*128+p, e], so each
    # partition's SBUF row is one contiguous DRAM run
    wqkt_bf = np.ascontiguousarray(
        wqkt.reshape(DT, P, 2 * E).transpose(1, 0, 2).reshape(P, DT * 2 * E)
    ).astype(ml_dtypes.bfloat16)
    # far K projection weights: W_k^T * KSCALE as fp8 hi+lo planes,
    # packed (p, o, plane, e)
    wkt = wqkt[:, E:]  # already scaled by KSCALE
    wkt_hi = wkt.astype(ml_dtypes.float8_e4m3)
    wkt_lo = (wkt - wkt_hi.astype(np.float32)).astype(ml_dtypes.float8_e4m3)
    wktf = np.ascontiguousarray(
        np.stack([wkt_hi, wkt_lo], axis=1)       # [D, 2, E]
        .reshape(DT, P, 2, E).transpose(1, 0, 2, 3).reshape(P, DT * 2 * E)
    )
    wvt = np.ascontiguousarray(
        np.asarray(W_v, dtype=np.float32).T
    ).astype(ml_dtypes.bfloat16)

    idx_np = np.zeros((16, 9), dtype=np.int16)
    idx_np[1:, 0] = -1  # scatter: slot 0 -> row 0, rest ignored

    in_maps = []
    for c in range(NCORES):
        b, h = divmod(c, 2)
        xt_b = x[b].T                                    # (D, N)
        near = xt_b[:, h * NH:(h + 1) * NH]
        far = xt_b[:, (1 - h) * NH:(2 - h) * NH]
        xt_c = np.ascontiguousarray(near).astype(ml_dtypes.bfloat16)
        xtf_c = np.ascontiguousarray(far).astype(ml_dtypes.float8_e4m3)
        at_b = adj[b].T[:, h * NH:(h + 1) * NH]          # (N m-rows, NH cols)
        if h == 1:
            at_c = np.concatenate([at_b[NH:], at_b[:NH]], axis=0)
        else:
            at_c = at_b
        at_c = np.ascontiguousarray(at_c).astype(ml_dtypes.float8_e4m3)
        im = {
            "xt": xt_c, "xtf": xtf_c, "at": at_c,
            "wqkt": wqkt_bf, "wktf": wktf, "wvt": wvt,
        }
        if CC_SWDGE:
            im["idxs"] = idx_np
        in_maps.append(im)
    return in_maps


def kernel(x, adj_matrix, W_qk, W_v):
    in_maps = prepare_in_maps(x, adj_matrix, W_qk, W_v)
    nc = _build()
    import os

    trace = os.environ.get("CAMIL_TRACE") == "1"
    kwargs = {}
    if trace:
        kwargs = {"trace": True, "trace_cores": list(range(NCORES))}
    res = run_bass_kernel_spmd(nc, in_maps, core_ids=list(range(NCORES)), **kwargs)

    global LAST_EXEC_NS, LAST_TRACE
    LAST_EXEC_NS = res.exec_time_ns
    LAST_TRACE = res.instructions_and_trace[1] if res.instructions_and_trace else None

    out = np.empty((B, N, D), dtype=np.float32)
    for c in range(NCORES):
        b, h = divmod(c, 2)
        out[b, h * NH:(h + 1) * NH] = res.results[c]["out"].astype(np.float32)
    return out


LAST_EXEC_NS = None
LAST_TRACE = None


# revision 60
# speedup vs baseline: 1.0031x; 1.0031x over previous
"""CAMIL self-attention kernel for 8 Trainium2 NeuronCores.

Reference computation (per bag b of B=4, N=4096 instances, D=512 features):
    qk = x @ W_qk.T ; q, k = split(qk)          (att dim E=64)
    v  = x @ W_v.T
    logits_n = (1/8) * sum_m adj[n,m] * (q_n . k_m)
             = (q_n/8) . (adj @ k)_n
    w = softmax(logits over N) ; out = w * v

Sharding: 2 cores per bag, each core owns NH=2048 rows (query dim). The
adjacency is pre-rotated on host so every core's rows are local m-tiles
0..15; the softmax normalization is completed with a tiny AllReduce of the
local sumexp between the two cores of each bag.

Numerics / dataflow (all tuned against the TimelineSim cost model, where
DMA payloads serialize on one exclusive device at ~360 B/ns and matmul cost
is output-free-size x cycles-per-row):
- The core loads only its own half of x in bf16 (Q, K-near, V); the other
  half is needed only for K, which tolerates fp8, so it ships as
  single-plane fp8 e4m3 (half the bytes). The far K projection runs
  entirely in fp8 (W_k^T staged as fp8 hi+lo planes, scaled x32 so the
  planes stay in e4m3 normal range; the 1/32 is folded into W_q together
  with the 1/sqrt(E) attention scale).
- QK uses x as the stationary operand so q/k come out of PSUM row-major -
  no PE transposes. Both S2 halves accumulate into one PSUM supertile
  consumed by a handful of big DVE/ACT ops: per-op fixed SBUF/PSUM access
  latency (~250-370ns) would otherwise pace the pipeline at ~2x the matmul
  rate.
- K is split on-device into fp8 hi+lo planes packed side-by-side in the
  DoubleRow stationary operand so a single pass over the adjacency computes
  both planes into PSUM partitions 0:64 (hi) and 64:128 (lo) at the fp8
  DoubleRow rate. The adjacency is exact in fp8 (binary) and streams
  n-block-major (4 column blocks of 512 rows, 512KB sub-DMAs) so each
  block's logits, exp and sum-partial complete while later blocks are still
  in flight. V-projection matmuls pace 1:1 with the stream subs (4 run
  pre-stream) so the PE never falls behind the adjacency DMA.
- Logit dot + softmax in fp32 on DVE/ACT; a fixed shift exp(l - 80) keeps
  exp in fp32 range (logits ~N(0,20), per-bag max ~75-100) and removes the
  serial max-reduction from the critical path. The last block's tail is the
  exposed critical path: its PSUM read is one full-width DVE copy (measured
  faster than DVE+ACT or DVE+DVE half-splits: ACT dispatches ~0.9us late
  after an idle period, and split copies pay the fixed PSUM access latency
  twice while the z-matmuls gate on the last half anyway).
- w-scales all run on DVE (194ns vs 612ns on ACT) and the output is stored
  bf16 (half the store bytes, upcast to fp32 on host) in packs of
  [2,2,4,4,4] row-tiles so the first store launches after two scales while
  later, larger packs keep HWDGE issue off the critical path.
"""

import sys

sys.path.insert(0, "/opt/trn_rl_repo")

import numpy as np

import concourse.bass as bass
import concourse.tile as tile
from concourse import bacc, bass_isa, mybir
from concourse.bass_utils import run_bass_kernel_spmd
from concourse.masks import make_identity

B, N, D, E = 4, 4096, 512, 64
P = 128
NCORES = 8
NH = N // 2        # rows per core
TH = NH // P       # 16 row-tiles per core (own half)
DT = D // P        # 4 d-tiles
KSCALE = 32.0      # K-path scale so fp8 W_k planes stay in e4m3 normal range
F32 = mybir.dt.float32
BF16 = mybir.dt.bfloat16
F8 = mybir.dt.float8e4
DR = mybir.MatmulPerfMode.DoubleRow
GROUPS = [[0, 1], [2, 3], [4, 5], [6, 7]]
I16 = mybir.dt.int16
# sumexp exchange via SWDGE prepare/trigger was attempted (descriptors
# prepared mid-stream, fired with cheap Pool-SEQ triggers to skip the two
# ~1.3us HWDGE+DGE issue latencies on the softmax tail) but the Tile
# framework's swdge semaphore management deadlocks the Pool queue; keep the
# plain-DMACopy exchange.
CC_SWDGE = False


def _build(single=False):
    # single=True: replace the cross-core AllReduce with a local DMA so the
    # module has no collectives (for TimelineSim cost modeling only).
    nc = bacc.Bacc(
        "TRN2", target_bir_lowering=False, num_devices=NCORES,
        num_swdge_queues=2 if CC_SWDGE else 1,
    )

    xt = nc.dram_tensor("xt", [D, NH], BF16, kind="ExternalInput")
    xtf = nc.dram_tensor("xtf", [D, NH], F8, kind="ExternalInput")
    at = nc.dram_tensor("at", [N, NH], F8, kind="ExternalInput")
    # weights staged partition-major on host so each partition's row is one
    # contiguous >=512B run (sub-512B runs pay a 2x DMA penalty)
    wqkt = nc.dram_tensor("wqkt", [P, DT * 2 * E], BF16, kind="ExternalInput")
    wktf = nc.dram_tensor("wktf", [P, DT * 2 * E], F8, kind="ExternalInput")
    wvt = nc.dram_tensor("wvt", [D, D], BF16, kind="ExternalInput")
    if CC_SWDGE:
        # swdge index patterns: col 0 = scatter ([0, -1 x15]), cols 1:9 =
        # gather (128 zeros, one per destination partition)
        idxs = nc.dram_tensor("idxs", [16, 9], I16, kind="ExternalInput")
    out = nc.dram_tensor("out", [NH, D], BF16, kind="ExternalOutput")

    xt_v = xt.ap().rearrange("(o p) n -> p o n", p=P)        # [128, 4, 2048]
    xtf_v = xtf.ap().rearrange("(o p) n -> p o n", p=P)      # [128, 4, 2048]
    at_v = at.ap().rearrange("(mo p) n -> p mo n", p=P)      # [128, 32, 2048]
    wqkt_v = wqkt.ap().rearrange("p (o e) -> p o e", o=DT)   # [128, 4, 128]
    wktf_v = wktf.ap().rearrange("p (o l e) -> p o l e", o=DT, l=2)
    wvt_v = wvt.ap().rearrange("(o p) e -> p o e", p=P)      # [128, 4, 512]
    out_v = out.ap().rearrange("(t p) e -> p t e", p=P)      # [128, 16, 512]

    with tile.TileContext(nc) as tc:
        with tc.tile_pool(name="big", bufs=1) as big, \
             tc.tile_pool(name="atp", bufs=16) as atp, \
             tc.tile_pool(name="ostream", bufs=5) as ostream, \
             tc.tile_pool(name="small", bufs=2) as small, \
             tc.tile_pool(name="dram", bufs=1, space="DRAM") as dram:

            # ---- constants ----
            # stacked double identity [I64; I64]: rt.T @ dident fuses the
            # z^T transpose with the hi+lo plane sum in a single PE op
            dident = big.tile([P, E], BF16)
            make_identity(nc, dident[0:E, 0:E])
            make_identity(nc, dident[E:2 * E, 0:E])
            # touch Exp once so the ACT table load is off the softmax path
            warm = small.tile([1, 1], F32, tag="warm")
            nc.gpsimd.memset(warm[:], 0.0)
            nc.scalar.activation(
                warm[:], warm[:], mybir.ActivationFunctionType.Exp
            )
            LSHIFT = 80.0
            nshift = small.tile([P, 1], F32, tag="nshift")
            nc.gpsimd.memset(nshift[:], -LSHIFT)

            # ---- input DMAs, all issued up front on the SP queue ----
            wqkt_sb = big.tile([P, DT, 2 * E], BF16)
            nc.sync.dma_start(out=wqkt_sb[:], in_=wqkt_v)
            wktf_sb = big.tile([P, DT, 2, E], F8, tag="wktf")
            nc.sync.dma_start(out=wktf_sb[:], in_=wktf_v)

            xt_q = []
            for j in range(4):
                xq = big.tile([P, DT, 512], BF16, tag=f"xt_q{j}")
                nc.sync.dma_start(
                    out=xq[:], in_=xt_v[:, :, j * 512:(j + 1) * 512]
                )
                xt_q.append(xq)

            xf_h = []
            for j in range(2):
                xf = big.tile([P, DT, 1024], F8, tag=f"xf_h{j}")
                nc.sync.dma_start(
                    out=xf[:], in_=xtf_v[:, :, j * 1024:(j + 1) * 1024]
                )
                xf_h.append(xf)

            wvt_sb = big.tile([P, DT, D], BF16, tag="wvt")
            nc.sync.dma_start(out=wvt_sb[:], in_=wvt_v)

            if CC_SWDGE:
                # swdge sumexp-exchange staging: index patterns + zeroed DRAM
                # landing pads (scatter-ADD needs a zero base). Tiny DMAs,
                # issued before the at stream so they are done by tail time.
                idx_sb = small.tile([16, 9], I16, tag="idx_sb")
                nc.sync.dma_start(out=idx_sb[:], in_=idxs.ap())
                s_pad = small.tile([P, 64], F32, tag="s_pad")
                nc.gpsimd.memset(s_pad[:], 0.0)
                gath_pad = small.tile([P, 1, 64], F32, tag="gath_pad")
                zq = small.tile([1, 64], F32, tag="zq")
                nc.gpsimd.memset(zq[:], 0.0)
                cc_pad = dram.tile([1, 64], F32)
                cc_out_pad = dram.tile([1, 64], F32)
                nc.sync.dma_start(out=cc_pad[:], in_=zq[:])
                nc.sync.dma_start(out=cc_out_pad[:], in_=zq[:])

            # n-block-major adjacency stream: block j = all 32 m-tiles x
            # cols j*512..(j+1)*512, in 4 sub-DMAs of 8 m-tiles each. Each
            # block's logits complete while later blocks still stream.
            at_tiles = {}
            for j in range(4):
                for s in range(4):
                    if (j, s) == (3, 3):
                        # final sub split 6+2 m-tiles (separate tiles): only
                        # one DoubleRow pair gates on the last DMA's 900ns
                        # completion sem
                        ta = big.tile([P, 6, 512], F8, tag="at33a")
                        tb = big.tile([P, 2, 512], F8, tag="at33b")
                        nc.sync.dma_start(
                            out=ta[:],
                            in_=at_v[:, 8 * s:8 * s + 6,
                                     j * 512:(j + 1) * 512],
                        )
                        nc.sync.dma_start(
                            out=tb[:],
                            in_=at_v[:, 8 * s + 6:8 * s + 8,
                                     j * 512:(j + 1) * 512],
                        )
                        at_tiles[(j, s)] = (ta, tb)
                        continue
                    t = atp.tile([P, 8, 512], F8, tag="at_t",
                                 name=f"at{j}_{s}")
                    nc.sync.dma_start(
                        out=t[:],
                        in_=at_v[:, 8 * s:8 * s + 8, j * 512:(j + 1) * 512],
                    )
                    at_tiles[(j, s)] = t

            # prepare the sumexp-exchange descriptors now (desc-gen runs on
            # Pool while the stream loads); the tail just fires triggers.
            if CC_SWDGE:
                cc_dsem = nc.alloc_semaphore("cc_dsem")
                bc_dsem = nc.alloc_semaphore("bc_dsem")
                nc.gpsimd.dma_scatter_add(
                    out_ap=cc_pad[:],
                    in_ap=s_pad[:].unsqueeze(1),
                    idxs_ap=idx_sb[:, 0:1],
                    num_idxs=16,
                    num_idxs_reg=16,
                    elem_size=64,
                    prepare_only=True,
                    sem=cc_dsem,
                    queue_num=0,
                )
                gath_src = cc_pad if single else cc_out_pad
                nc.gpsimd.dma_gather(
                    out_ap=gath_pad[:],
                    in_ap=gath_src[:],
                    idxs_ap=idx_sb[:, 1:9],
                    num_idxs=128,
                    num_idxs_reg=128,
                    elem_size=64,
                    prepare_only=True,
                    sem=bc_dsem,
                    queue_num=1,
                )

            q_nat = big.tile([P, TH, E], BF16)
            # K for all 32 m-tiles in one tile, packed (hi 0:E | lo E:2E) per
            # m-tile; tiles 0..15 = own half (bf16 path), 16..31 = far (fp8)
            k_all = big.tile([P, 2 * TH, 2 * E], F8, tag="k_all")
            v_ev = big.tile([P, TH // 2, D], BF16, tag="v_ev")
            v_od = big.tile([P, TH // 2, D], BF16, tag="v_od")
            rt_c = [
                big.tile([P, 512], BF16, tag=f"rt{rc}", name=f"rt{rc}")
                for rc in range(4)
            ]
            l_sb = big.tile([P, TH], F32)

            # ---- S2: QK projections, x stationary so q/k land row-major.
            # All 16 tiles of each half accumulate into one PSUM supertile,
            # consumed by a handful of big DVE/ACT ops: the per-op fixed
            # SBUF/PSUM access latency (~250-370ns) would otherwise pace the
            # whole pipeline at ~2x the matmul rate. ----
            with tc.tile_pool(name="ps_a", bufs=1, space="PSUM") as ps_a, \
                 tc.tile_pool(name="ps_b", bufs=1, space="PSUM") as ps_b:
                # near (own) half: bf16, fused Q|K -> [n, 128] per tile
                pqk = ps_a.tile([P, TH, 2 * E], F32)
                for t in range(TH):
                    xo = (t % 4) * P
                    for di in range(DT):
                        nc.tensor.matmul(
                            pqk[:, t, :],
                            xt_q[t // 4][:, di, xo:xo + P],
                            wqkt_sb[:, di, :],
                            start=(di == 0),
                            stop=(di == DT - 1),
                        )
                nc.scalar.copy(out=q_nat[:], in_=pqk[:, :, 0:E])
                nc.vector.tensor_copy(
                    out=k_all[:, 0:TH, 0:E], in_=pqk[:, :, E:2 * E]
                )
                nc.vector.tensor_tensor(
                    out=k_all[:, 0:TH, E:2 * E],
                    in0=pqk[:, :, E:2 * E],
                    in1=k_all[:, 0:TH, 0:E],
                    op=mybir.AluOpType.subtract,
                )
                # far half: all-fp8 K-only projection, W_k hi+lo planes
                pkf = ps_b.tile([P, TH, E], F32)
                for t in range(TH):
                    xo = (t % 8) * P
                    for di in range(DT):
                        for pl in range(2):
                            nc.tensor.matmul(
                                pkf[:, t, :],
                                xf_h[t // 8][:, di, xo:xo + P],
                                wktf_sb[:, di, pl, :],
                                start=(di == 0 and pl == 0),
                                stop=(di == DT - 1 and pl == 1),
                            )
                nc.scalar.copy(out=k_all[:, TH:2 * TH, 0:E], in_=pkf[:])
                nc.vector.tensor_tensor(
                    out=k_all[:, TH:2 * TH, E:2 * E],
                    in0=pkf[:],
                    in1=k_all[:, TH:2 * TH, 0:E],
                    op=mybir.AluOpType.subtract,
                )

            # ---- V projection interleaved 1:1 with S4 DoubleRow pairs ----
            # S4: R^T = (adj @ [K_hi K_lo])^T. lhsT [128, 2, 128]: m-tile
            # pair, cols = (hi 64 | lo 64); rhs [128, 2, 512]: same pair of
            # adjacency m-tiles. One pass over adj fills PSUM rows 0:64 (hi)
            # and 64:128 (lo). V-tile t fills the PE while at-tile t+1 lands.
            with tc.tile_pool(name="ps_r", bufs=1, space="PSUM") as ps_r, \
                 tc.tile_pool(name="ps_s", bufs=2, space="PSUM") as ps_s:
                with tc.tile_pool(name="ps_v", bufs=2, space="PSUM") as ps_v:
                    # one psum tile per n-block; each block's tail (psum
                    # read, z-dot, exp partial) runs while later blocks
                    # still stream in.
                    psum_rj = [
                        ps_r.tile([P, 512], F32, tag=f"pr{j}", name=f"pr{j}")
                        for j in range(4)
                    ]
                    e_sb = small.tile([P, TH], F32, tag="e_sb")
                    s_p4 = small.tile([P, 4], F32, tag="s_p4")
                    def v_tile(t):
                        psum_v = ps_v.tile([P, 512], F32, tag="pv",
                                           name=f"psv{t}")
                        xr = xt_q[t // 4]
                        xo = (t % 4) * P
                        for di in range(DT):
                            nc.tensor.matmul(
                                psum_v[:],
                                xr[:, di, xo:xo + P],
                                wvt_sb[:, di, :],
                                start=(di == 0),
                                stop=(di == DT - 1),
                            )
                        if t % 2 == 0:
                            nc.vector.tensor_copy(
                                out=v_ev[:, t // 2, :], in_=psum_v[:]
                            )
                        else:
                            nc.scalar.copy(out=v_od[:, t // 2, :], in_=psum_v[:])

                    # 4 V tiles run pre-stream (right after wvt lands), the
                    # remaining 12 pace 1:1 with the first 12 adjacency subs
                    # so the PE never falls behind the at stream.
                    for t in range(4):
                        v_tile(t)

                    for j in range(4):
                        for s in range(4):
                            si = 4 * j + s
                            if si < 12:
                                v_tile(4 + si)

                            a_t = at_tiles[(j, s)]
                            for gi in range(4):
                                g = 4 * s + gi
                                kt = k_all[:, 2 * g:2 * g + 2, :]
                                if isinstance(a_t, tuple):
                                    if gi < 3:
                                        src = a_t[0][:, 2 * gi:2 * gi + 2, :]
                                    else:
                                        src = a_t[1][:, 0:2, :]
                                else:
                                    src = a_t[:, 2 * gi:2 * gi + 2, :]
                                nc.tensor.matmul(
                                    psum_rj[j][:],
                                    kt,
                                    src,
                                    start=(s == 0 and gi == 0),
                                    stop=(s == 3 and gi == 3),
                                    perf_mode=DR,
                                    skip_group_check=True,
                                )

                        # ---- block-j tail: psum -> SBUF, z = hi+lo via
                        # double-identity matmul, l = q.z, exp partial ----
                        # split the psum read across DVE and ACT so the
                        # serial tail of the last block is halved
                        nc.vector.tensor_copy(
                            out=rt_c[j][:], in_=psum_rj[j][:]
                        )
                        zp4 = ps_s.tile([P, 4, E], F32, tag="ps",
                                        name=f"z5_{j}")
                        z4 = small.tile([P, 4, E], BF16, tag="z4",
                                        name=f"z4_{j}")
                        for i in range(4):
                            nc.tensor.matmul(
                                zp4[:, i, :],
                                rt_c[j][:, i * P:(i + 1) * P],
                                dident[:],
                                start=True,
                                stop=True,
                            )
                        nc.vector.tensor_tensor(
                            out=z4[:], in0=zp4[:],
                            in1=q_nat[:, j * 4:(j + 1) * 4, :],
                            op=mybir.AluOpType.mult,
                        )
                        nc.vector.tensor_reduce(
                            out=l_sb[:, j * 4:(j + 1) * 4], in_=z4[:],
                            axis=mybir.AxisListType.X, op=mybir.AluOpType.add,
                        )
                        nc.scalar.activation(
                            e_sb[:, j * 4:(j + 1) * 4],
                            l_sb[:, j * 4:(j + 1) * 4],
                            mybir.ActivationFunctionType.Exp,
                            bias=nshift[:, 0:1], scale=1.0,
                            accum_out=s_p4[:, j:j + 1],
                        )
                s_loc = small.tile([P, 1], F32, tag="s_loc")
                nc.vector.tensor_reduce(
                    out=s_loc[:], in_=s_p4[:],
                    axis=mybir.AxisListType.X, op=mybir.AluOpType.add,
                )
                s_red = small.tile([P, 1], F32, tag="s_red")
                nc.gpsimd.partition_all_reduce(
                    s_red[:], s_loc[:], channels=P,
                    reduce_op=bass_isa.ReduceOp.add,
                )

                s_inv = small.tile([P, 1], F32, tag="s_inv")
                if CC_SWDGE:
                    # scatter s_red -> DRAM, AllReduce, gather-broadcast back.
                    # Tile transfers the preps' data deps onto the triggers
                    # and attaches the SWDGE DMA sems to consumers itself.
                    nc.vector.tensor_copy(out=s_pad[:, 0:1], in_=s_red[:])
                    nc.gpsimd.trigger_dma(count=None, queue_num=0)
                    if not single:
                        nc.gpsimd.collective_compute(
                            "AllReduce",
                            mybir.AluOpType.add,
                            replica_groups=GROUPS,
                            ins=[cc_pad[0:1, 0:1].opt()],
                            outs=[cc_out_pad[0:1, 0:1].opt()],
                        )
                    nc.gpsimd.trigger_dma(count=None, queue_num=1)
                    nc.vector.reciprocal(s_inv[:], gath_pad[:, 0, 0:1])
                else:
                    cc_in = dram.tile([1, 1], F32)
                    cc_out = dram.tile([1, 1], F32)
                    nc.sync.dma_start(out=cc_in[:], in_=s_red[0:1, :])
                    if single:
                        cc_res = cc_in
                    else:
                        nc.gpsimd.collective_compute(
                            "AllReduce",
                            mybir.AluOpType.add,
                            replica_groups=GROUPS,
                            ins=[cc_in[:].opt()],
                            outs=[cc_out[:].opt()],
                        )
                        cc_res = cc_out
                    # broadcast-load the pair total to every partition
                    gath_bc = small.tile([P, 1], F32, tag="gath_bc")
                    nc.sync.dma_start(
                        out=gath_bc[:],
                        in_=cc_res[:].rearrange("a b -> (a b)").unsqueeze(0)
                        .broadcast_to((P, 1)),
                    )
                    nc.vector.reciprocal(s_inv[:], gath_bc[:])

                # w = exp(l - LSHIFT) / S
                w_sb = small.tile([P, TH], F32, tag="w_sb")
                nc.vector.tensor_scalar_mul(w_sb[:], e_sb[:], s_inv[:, 0:1])

                # ---- S7: scale V by w and store (bf16) ----
                # all scales on DVE (194ns each vs 612 on ACT); first store
                # pack is small so the store pipeline starts early, DVE then
                # stays ahead of the 1456ns/pack DMA pace
                PACKS = [2, 2, 4, 4, 4]
                t = 0
                for pi, pk in enumerate(PACKS):
                    o_t = ostream.tile(
                        [P, pk, D], BF16, tag=f"o_t{pk}", name=f"ot{pi}"
                    )
                    for i in range(pk):
                        src = (v_ev[:, (t + i) // 2, :] if (t + i) % 2 == 0
                               else v_od[:, (t + i) // 2, :])
                        nc.vector.tensor_scalar_mul(
                            o_t[:, i, :], src, w_sb[:, t + i:t + i + 1],
                        )
                    nc.sync.dma_start(
                        out=out_v[:, t:t + pk, :], in_=o_t[:]
                    )
                    t += pk

    nc.compile()
    return nc


def prepare_in_maps(x, adj_matrix, W_qk, W_v):
    import ml_dtypes

    x = np.asarray(x, dtype=np.float32)
    adj = np.asarray(adj_matrix, dtype=np.float32)
    wqkt = np.ascontiguousarray(np.asarray(W_qk, dtype=np.float32).T)
    wqkt = wqkt.copy()
    # fold attention scale and the 1/KSCALE K-plane scale into W_q
    wqkt[:, :E] *= 1.0 / (np.sqrt(E) * KSCALE)
    wqkt[:, E:] *= KSCALE
    # partition-major staging: element (p, o, e) = wqkt[o*128+p, e], so each
    # partition's SBUF row is one contiguous DRAM run
    wqkt_bf = np.ascontiguousarray(
        wqkt.reshape(DT, P, 2 * E).transpose(1, 0, 2).reshape(P, DT * 2 * E)
    ).astype(ml_dtypes.bfloat16)
    # far K projection weights: W_k^T * KSCALE as fp8 hi+lo planes,
    # packed (p, o, plane, e)
    wkt = wqkt[:, E:]  # already scaled by KSCALE
    wkt_hi = wkt.astype(ml_dtypes.float8_e4m3)
    wkt_lo = (wkt - wkt_hi.astype(np.float32)).astype(ml_dtypes.float8_e4m3)
    wktf = np.ascontiguousarray(
        np.stack([wkt_hi, wkt_lo], axis=1)       # [D, 2, E]
        .reshape(DT, P, 2, E).transpose(1, 0, 2, 3).reshape(P, DT * 2 * E)
    )
    wvt = np.ascontiguousarray(
        np.asarray(W_v, dtype=np.float32).T
    ).astype(ml_dtypes.bfloat16)

    idx_np = np.zeros((16, 9), dtype=np.int16)
    idx_np[1:, 0] = -1  # scatter: slot 0 -> row 0, rest ignored

    in_maps = []
    for c in range(NCORES):
        b, h = divmod(c, 2)
        xt_b = x[b].T                                    # (D, N)
        near = xt_b[:, h * NH:(h + 1) * NH]
        far = xt_b[:, (1 - h) * NH:(2 - h) * NH]
        xt_c = np.ascontiguousarray(near).astype(ml_dtypes.bfloat16)
        xtf_c = np.ascontiguousarray(far).astype(ml_dtypes.float8_e4m3)
        at_b = adj[b].T[:, h * NH:(h + 1) * NH]          # (N m-rows, NH cols)
        if h == 1:
            at_c = np.concatenate([at_b[NH:], at_b[:NH]], axis=0)
        else:
            at_c = at_b
        at_c = np.ascontiguousarray(at_c).astype(ml_dtypes.float8_e4m3)
        im = {
            "xt": xt_c, "xtf": xtf_c, "at": at_c,
            "wqkt": wqkt_bf, "wktf": wktf, "wvt": wvt,
        }
        if CC_SWDGE:
            im["idxs"] = idx_np
        in_maps.append(im)
    return in_maps


def kernel(x, adj_matrix, W_qk, W_v):
    in_maps = prepare_in_maps(x, adj_matrix, W_qk, W_v)
    nc = _build()
    import os

    trace = os.environ.get("CAMIL_TRACE") == "1"
    kwargs = {}
    if trace:
        kwargs = {"trace": True, "trace_cores": list(range(NCORES))}
    res = run_bass_kernel_spmd(nc, in_maps, core_ids=list(range(NCORES)), **kwargs)

    global LAST_EXEC_NS, LAST_TRACE
    LAST_EXEC_NS = res.exec_time_ns
    LAST_TRACE = res.instructions_and_trace[1] if res.instructions_and_trace else None

    out = np.empty((B, N, D), dtype=np.float32)
    for c in range(NCORES):
        b, h = divmod(c, 2)
        out[b, h * NH:(h + 1) * NH] = res.results[c]["out"].astype(np.float32)
    return out


LAST_EXEC_NS = None
LAST_TRACE = None
